# revision 11
# baseline (speedup 1.0000x reference)
import sys

sys.path.insert(0, "/opt/trn_rl_repo")

import ml_dtypes
import numpy as np

import bass_rust
import concourse.bass as bass
import concourse.mybir as mybir
import concourse.tile as tile
from concourse import bass_utils
from concourse.tile import ScopedClock

B, T, C = 4, 2048, 1024
H, HD = 16, 64
HPC = 8
GC = HPC * HD
QB = 512
KBLK = 128
NQC = T // QB
NKT = T // KBLK
KT = C // 128

F32 = mybir.dt.float32
BF16 = mybir.dt.bfloat16
BF16NP = ml_dtypes.bfloat16


_MAX_WAITS = 1


def _split_multi_waits(nc: bass.Bass) -> None:
    eng_by_type = nc.engines

    n_es = [0]

    def make_nop(engine_type, wait):
        eng = eng_by_type[engine_type]
        if engine_type == mybir.EngineType.Pool:
            inst = mybir.InstEventSemaphore(
                name=f"I-wsplit-es-{n_es[0]}", ins=[], outs=[]
            )
            n_es[0] += 1
            inst.engine = engine_type
            inst.sync_info = bass_rust.SyncInfo(on_wait=[wait], on_update=[])
            return inst
        binst = eng.nop(hint="wsplit", nofuse=True)
        cur = nc.cur_bb.bb
        insts = list(cur.instructions)
        assert insts and insts[-1] is binst.ins
        cur.instructions = insts[:-1]
        binst.ins.sync_info = bass_rust.SyncInfo(on_wait=[wait], on_update=[])
        return binst.ins

    for f in nc.m.functions:
        for bb in f.blocks:
            changed = False
            new_insts = []
            for inst in bb.instructions:
                si = inst.sync_info
                waits = list(si.on_wait) if si is not None and si.on_wait else []
                if len(waits) > _MAX_WAITS:
                    for w in waits[:-_MAX_WAITS]:
                        new_insts.append(make_nop(inst.engine, w))
                    si.on_wait = waits[-_MAX_WAITS:]
                    changed = True
                new_insts.append(inst)
            if changed:
                bb.instructions = new_insts


def _drain_and_barrier_split(self, tick_clock, wait_clock):
    nc = self.nc
    drain_inst = nc.sync.drain()
    wait_clock.add_sem_waits(
        drain_inst.ins, ScopedClock({None: tick_clock.global_clock})
    )
    nc.all_engine_barrier()
    assert self.sems is not None
    popped = nc._tile_sem_poison_stack.pop()
    assert popped is self._sem_poison
    nc.clear_and_free_semaphores(list(self.sems.allocated().values()))
    nc.all_engine_barrier()
    _split_multi_waits(nc)


tile.TileContext._drain_and_barrier = _drain_and_barrier_split


def build_nc(with_bias: bool) -> bass.Bass:
    nc = bass.Bass("TRN2", target_bir_lowering=False)

    xT = nc.declare_dram_parameter("xT", [C, T], BF16, isOutput=False)
    wqk = nc.declare_dram_parameter("wqk", [C, 2 * GC], BF16, isOutput=False)
    wv = nc.declare_dram_parameter("wv", [C, GC], BF16, isOutput=False)
    wp = nc.declare_dram_parameter("wp", [GC, C], BF16, isOutput=False)
    maskp = nc.declare_dram_parameter("mask", [128, 4 * QB], BF16, isOutput=False)
    if with_bias:
        bqk = nc.declare_dram_parameter("bqk", [1, 2 * GC], BF16, isOutput=False)
        bv = nc.declare_dram_parameter("bv", [1, GC], BF16, isOutput=False)
    out = nc.declare_dram_parameter("out", [T, C], F32, isOutput=True)

    with tile.TileContext(nc) as tc:
        with (
            tc.tile_pool(name="singles", bufs=1) as singles,
            tc.tile_pool(name="exp", bufs=6) as exp_pool,
            tc.tile_pool(name="small", bufs=4) as small_pool,
            tc.tile_pool(name="outsb", bufs=3) as out_pool,
            tc.tile_pool(name="dram", bufs=4, space="DRAM") as dram_pool,
            tc.tile_pool(name="ps", bufs=2, space="PSUM") as ps_pool,
            tc.tile_pool(name="ps_att", bufs=3, space="PSUM") as ps_att_pool,
            tc.tile_pool(name="ps_y", bufs=2, space="PSUM") as ps_y_pool,
        ):
            xT_sb = singles.tile([128, KT, T], BF16, tag="xT")
            wqk_sb = singles.tile([128, KT, 2 * GC], BF16, tag="wqk")
            wv_sb = singles.tile([128, KT, GC], BF16, tag="wv")
            wp_sb = singles.tile([128, 4, C], BF16, tag="wp")
            mask_sb = singles.tile([128, 4 * QB], BF16, tag="mask")
            qkT_sb = singles.tile([128, 8, T], BF16, tag="qkT")
            vv_sb = singles.tile([128, HPC, NKT, HD + 1], BF16, tag="vv")
            yTn_sb = singles.tile([128, 4, T], BF16, tag="yTn")

            nc.sync.dma_start(
                out=xT_sb[:], in_=xT.rearrange("(kt p) t -> p kt t", p=128)
            )
            nc.sync.dma_start(
                out=wqk_sb[:], in_=wqk.rearrange("(kt p) m -> p kt m", p=128)
            )
            nc.sync.dma_start(
                out=wv_sb[:], in_=wv.rearrange("(kt p) m -> p kt m", p=128)
            )
            nc.sync.dma_start(
                out=wp_sb[:], in_=wp.rearrange("(ct p) m -> p ct m", p=128)
            )
            nc.sync.dma_start(out=mask_sb[:], in_=maskp[:, :])
            if with_bias:
                bqk_sb = singles.tile([1, 2 * GC], BF16, tag="bqk")
                bv_sb = singles.tile([1, GC], BF16, tag="bv")
                ones_sb = singles.tile([1, T], BF16, tag="ones")
                nc.sync.dma_start(out=bqk_sb[:], in_=bqk[:, :])
                nc.sync.dma_start(out=bv_sb[:], in_=bv[:, :])
                nc.vector.memset(ones_sb[:], 1.0)

            nc.vector.memset(vv_sb[:, :, :, HD], 1.0)

            for mt in range(8):
                for ntc in range(NQC):
                    ps = ps_pool.tile([128, QB], F32, tag="ps")
                    for kt in range(KT):
                        nc.tensor.matmul(
                            ps[:],
                            lhsT=wqk_sb[:, kt, mt * 128 : (mt + 1) * 128],
                            rhs=xT_sb[:, kt, ntc * QB : (ntc + 1) * QB],
                            start=(kt == 0),
                            stop=(kt == KT - 1 and not with_bias),
                        )
                    if with_bias:
                        nc.tensor.matmul(
                            ps[:],
                            lhsT=bqk_sb[0:1, mt * 128 : (mt + 1) * 128],
                            rhs=ones_sb[0:1, ntc * QB : (ntc + 1) * QB],
                            start=False,
                            stop=True,
                        )
                    nc.vector.tensor_copy(
                        qkT_sb[:, mt, ntc * QB : (ntc + 1) * QB], ps[:]
                    )

            for tt in range(NKT):
                ps = ps_pool.tile([128, QB], F32, tag="ps")
                for kt in range(KT):
                    nc.tensor.matmul(
                        ps[:],
                        lhsT=xT_sb[:, kt, tt * 128 : (tt + 1) * 128],
                        rhs=wv_sb[:, kt, :],
                        start=(kt == 0),
                        stop=(kt == KT - 1 and not with_bias),
                    )
                if with_bias:
                    nc.tensor.matmul(
                        ps[:],
                        lhsT=ones_sb[0:1, tt * 128 : (tt + 1) * 128],
                        rhs=bv_sb[0:1, :],
                        start=False,
                        stop=True,
                    )
                nc.vector.tensor_copy(
                    vv_sb[:, :, tt, 0:HD],
                    ps[:].rearrange("p (h d) -> p h d", h=HPC),
                )

            for h in range(HPC):
                prt = 64 * (h % 2)
                mtq = h // 2
                mtk = 4 + h // 2
                for qc in range(NQC):
                    nkb = 4 * (qc + 1)
                    ps_y = ps_y_pool.tile([HD + 1, QB], F32, tag="ps_y")
                    for kb in range(nkb):
                        ps_att = ps_att_pool.tile([128, QB], F32, tag="ps_att")
                        nc.tensor.matmul(
                            ps_att[:],
                            lhsT=qkT_sb[
                                prt : prt + 64, mtk, kb * 128 : (kb + 1) * 128
                            ],
                            rhs=qkT_sb[prt : prt + 64, mtq, qc * QB : (qc + 1) * QB],
                            start=True,
                            stop=True,
                        )
                        exp_t = exp_pool.tile([128, QB], BF16, tag="exp")
                        nc.scalar.activation(
                            exp_t[:],
                            ps_att[:],
                            mybir.ActivationFunctionType.Exp,
                            scale=0.125,
                        )
                        if kb >= 4 * qc:
                            m = kb - 4 * qc
                            nc.vector.tensor_mul(
                                exp_t[:],
                                exp_t[:],
                                mask_sb[:, m * QB : (m + 1) * QB],
                            )
                        nc.tensor.matmul(
                            ps_y[:],
                            lhsT=vv_sb[:, h, kb, :],
                            rhs=exp_t[:],
                            start=(kb == 0),
                            stop=(kb == nkb - 1),
                        )
                    recip = small_pool.tile([1, QB], F32, tag="recip")
                    nc.vector.reciprocal(recip[:], ps_y[HD : HD + 1, :])
                    recip_dram = dram_pool.tile([1, QB], F32, tag="recip_dram")
                    nc.sync.dma_start(out=recip_dram[:], in_=recip[:])
                    bcast = small_pool.tile([64, QB], F32, tag="bcast")
                    nc.sync.dma_start(
                        out=bcast[:], in_=recip_dram[:].to_broadcast((64, QB))
                    )
                    nc.vector.tensor_mul(
                        yTn_sb[prt : prt + 64, h // 2, qc * QB : (qc + 1) * QB],
                        ps_y[0:HD, :],
                        bcast[:],
                    )

            for tt in range(NKT):
                out_sb = out_pool.tile([128, C], F32, tag="out_sb")
                for nt2 in range(2):
                    ps = ps_pool.tile([128, QB], F32, tag="ps")
                    for ct in range(4):
                        nc.tensor.matmul(
                            ps[:],
                            lhsT=yTn_sb[:, ct, tt * 128 : (tt + 1) * 128],
                            rhs=wp_sb[:, ct, nt2 * QB : (nt2 + 1) * QB],
                            start=(ct == 0),
                            stop=(ct == 3),
                        )
                    nc.vector.tensor_copy(
                        out_sb[:, nt2 * QB : (nt2 + 1) * QB], ps[:]
                    )
                nc.sync.dma_start(
                    out=out[tt * 128 : (tt + 1) * 128, :], in_=out_sb[:]
                )

    return nc


def _make_mask() -> np.ndarray:
    p = np.arange(128)[:, None]
    i = np.arange(QB)[None, :]
    blocks = [(p + 128 * m <= i) for m in range(4)]
    return np.concatenate(blocks, axis=1).astype(BF16NP)


_NC_CACHE: dict[bool, bass.Bass] = {}


def kernel(x, w_qkv, b_qkv, w_proj, b_proj):
    x = np.asarray(x, dtype=np.float32)
    w_qkv = np.asarray(w_qkv, dtype=np.float32)
    b_qkv = np.asarray(b_qkv, dtype=np.float32)
    w_proj = np.asarray(w_proj, dtype=np.float32)
    b_proj = np.asarray(b_proj, dtype=np.float32)

    with_bias = bool(np.any(b_qkv))
    if with_bias not in _NC_CACHE:
        _NC_CACHE[with_bias] = build_nc(with_bias)
    nc = _NC_CACHE[with_bias]

    mask = _make_mask()
    in_maps = []
    for c in range(8):
        b, g = c // 2, c % 2
        cols = slice(g * GC, (g + 1) * GC)
        m = {
            "xT": np.ascontiguousarray(x[b].T).astype(BF16NP),
            "wqk": np.concatenate(
                [w_qkv[:, cols], w_qkv[:, C:][:, cols]], axis=1
            ).astype(BF16NP),
            "wv": np.ascontiguousarray(w_qkv[:, 2 * C :][:, cols]).astype(BF16NP),
            "wp": np.ascontiguousarray(w_proj[cols, :]).astype(BF16NP),
            "mask": mask,
        }
        if with_bias:
            m["bqk"] = np.concatenate([b_qkv[cols], b_qkv[C:][cols]])[None, :].astype(
                BF16NP
            )
            m["bv"] = b_qkv[2 * C :][cols][None, :].astype(BF16NP)
        in_maps.append(m)

    res = bass_utils.run_bass_kernel_spmd(nc, in_maps, core_ids=list(range(8)))

    out = np.empty((B, T, C), dtype=np.float32)
    for b in range(B):
        out[b] = res.results[2 * b]["out"] + res.results[2 * b + 1]["out"] + b_proj
    return out
